# revision 10
# baseline (speedup 1.0000x reference)
"""Trainium2 Bass kernel for a single causal attention head.

Problem: x:(8,2048,1024) f32, per-head projections wq/wk/wv:(64,1024),
biases (64,). Output: softmax(causal(q k^T / sqrt(64))) @ v : (8,2048,64).

Strategy:
  - Data-parallel: batch b -> core b (8 cores, 1 batch each).
  - Host prep: x[b] transposed to xT:(1024,2048) so the contraction dim (D)
    lands on SBUF partitions; weights pre-transposed to (1024,64); the
    1/sqrt(64) scale folded into wq/bq.
  - Device (per core):
      * qT/kT/vT = W^T.T @ xT  (PSUM accumulate over 8 d-tiles, fp32r matmuls)
      * vT transposed back to (T,64) tiles via PE transpose, augmented with a
        ones column (-> softmax denominator rides along the PV matmul).
      * scores computed transposed: S^T[j,i] = sum_h kT[h,j] qT[h,i] so that
        P^T has keys on partitions = contraction layout the PV matmul needs.
      * causal: k-tiles above the diagonal skipped entirely; diagonal-band
        tiles masked with a shifted triangular -1e9 mask (generated on device).
      * P^T = exp(S^T + mask) on the scalar (ACT) engine, PSUM -> SBUF.
      * O^T_aug[65, T] accumulated in PSUM over k-tiles; row 64 = sum_j P^T.
  - Host post: out[b] = (O^T[0:64] / O^T[64:65]).T  (softmax normalization).
"""

import numpy as np

B, T, D, HD = 8, 2048, 1024, 64
P = 128          # SBUF partitions
CH = 512         # q-chunk (matmul moving dim)
NCH = T // CH    # 4
DT = D // P      # 8 d-tiles
NKT = T // P     # 16 k-tiles
NEG = -1.0e9

_MM_DTYPE = "float16"    # matmul input dtype (1 cyc/row; 11-bit mantissa)

LAST_RESULTS = None      # BassKernelResults of the most recent run (for test.py)


def _build_module(legalize=True):
    import concourse.bass as bass
    import concourse.mybir as mybir
    from concourse.tile import TileContext
    from concourse.masks import make_identity

    F32 = mybir.dt.float32
    MMDT = getattr(mybir.dt, _MM_DTYPE)

    nc = bass.Bass("TRN2", target_bir_lowering=True)

    xT = nc.dram_tensor("xT", (D, T), MMDT, kind="ExternalInput")
    wqT = nc.dram_tensor("wqT", (D, HD), MMDT, kind="ExternalInput")
    wkT = nc.dram_tensor("wkT", (D, HD), MMDT, kind="ExternalInput")
    wvT = nc.dram_tensor("wvT", (D, HD), MMDT, kind="ExternalInput")
    bq = nc.dram_tensor("bq", (HD, 1), F32, kind="ExternalInput")
    bk = nc.dram_tensor("bk", (HD, 1), F32, kind="ExternalInput")
    bv = nc.dram_tensor("bv", (HD, 1), F32, kind="ExternalInput")
    outT = nc.dram_tensor("outT", (HD + 1, T), F32, kind="ExternalOutput")

    with TileContext(nc) as tc:
        with (
            tc.tile_pool(name="const", bufs=1) as const,
            tc.tile_pool(name="acts", bufs=1) as acts,
            tc.tile_pool(name="proj_ps", bufs=2, space="PSUM") as proj_ps,
            tc.tile_pool(name="tr_ps", bufs=1, space="PSUM") as tr_ps,
            tc.tile_pool(name="s_ps", bufs=3, space="PSUM") as s_ps,
            tc.tile_pool(name="o_ps", bufs=2, space="PSUM") as o_ps,
            tc.tile_pool(name="pwork", bufs=4) as pwork,
            tc.tile_pool(name="owork", bufs=2) as owork,
        ):
            # ---- constants / weights ----
            ident = const.tile([P, P], F32, name="ident")
            make_identity(nc, ident)

            # maskbig[p, c] = 0 if c >= p + 384 else NEG; diagonal-band tile
            # with offset delta = k_tile_start - q_chunk_start is the slice
            # maskbig[:, 384-delta : 896-delta].
            maskbig = const.tile([P, 896], F32, name="maskbig")
            nc.gpsimd.memset(maskbig[:], 0.0)
            nc.gpsimd.affine_select(
                out=maskbig[:], in_=maskbig[:],
                compare_op=mybir.AluOpType.is_ge, fill=NEG,
                base=-384, pattern=[[1, 896]], channel_multiplier=-1,
            )

            w_sb = {}
            for nm, dram in (("wq", wqT), ("wk", wkT), ("wv", wvT)):
                w = const.tile([P, DT, HD], MMDT, name=f"{nm}_sb")
                nc.sync.dma_start(out=w[:], in_=dram.rearrange("(n p) h -> p n h", p=P))
                w_sb[nm] = w
            b_sb = {}
            for nm, dram in (("bq", bq), ("bk", bk), ("bv", bv)):
                b = const.tile([HD, 1], F32, name=f"{nm}_sb")
                nc.sync.dma_start(out=b[:], in_=dram[:, :])
                b_sb[nm] = b

            xt = []
            for d in range(DT):
                t = const.tile([P, T], MMDT, name=f"xt{d}")
                nc.sync.dma_start(out=t[:], in_=xT[d * P:(d + 1) * P, :])
                xt.append(t)

            # ---- activations ----
            qT = acts.tile([HD, T], MMDT, name="qT")
            kT = acts.tile([HD, T], MMDT, name="kT")
            vT = acts.tile([HD, T], F32, name="vT")
            v_aug = acts.tile([P, NKT, HD + 1], MMDT, name="v_aug")
            nc.vector.memset(v_aug[:, :, HD], 1.0)

            # ---- projections: proj^T[h, t] = sum_d w^T[d, h] * xT[d, t] ----
            for ci in range(NCH):
                cs = slice(ci * CH, (ci + 1) * CH)
                for nm, dst in (("wq", qT), ("wk", kT), ("wv", vT)):
                    ps = proj_ps.tile([HD, CH], F32, name="proj", tag="proj")
                    for d in range(DT):
                        nc.tensor.matmul(
                            ps[:], w_sb[nm][:, d, :], xt[d][:, cs],
                            start=(d == 0), stop=(d == DT - 1),
                        )
                    nc.vector.tensor_scalar_add(dst[:, cs], ps[:], b_sb["b" + nm[1]][:])
                # vT chunk -> v_aug tiles (T on partitions) as soon as ready
                for tt in range(4 * ci, 4 * ci + 4):
                    tp = tr_ps.tile([P, HD], F32, name="vtr", tag="vtr")
                    nc.tensor.transpose(tp[:], vT[:, tt * P:(tt + 1) * P], ident[:HD, :HD])
                    nc.vector.tensor_copy(v_aug[:, tt, 0:HD], tp[:])

            # ---- attention, q-chunk at a time ----
            for ci in range(NCH):
                cs = slice(ci * CH, (ci + 1) * CH)
                nkt = 4 * (ci + 1)
                ops = o_ps.tile([HD + 1, CH], F32, name="oacc", tag="oacc")
                for kt in range(nkt):
                    sps = s_ps.tile([P, CH], F32, name="sT", tag="sT")
                    nc.tensor.matmul(
                        sps[:], kT[:, kt * P:(kt + 1) * P], qT[:, cs],
                        start=True, stop=True,
                    )
                    delta = kt * P - ci * CH
                    if delta >= 0:  # diagonal band: apply causal mask
                        nc.vector.tensor_tensor(
                            sps[:], sps[:], maskbig[:, 384 - delta:896 - delta],
                            mybir.AluOpType.add,
                        )
                    pt = pwork.tile([P, CH], MMDT, name="pT", tag="pT")
                    nc.scalar.activation(pt[:], sps[:], mybir.ActivationFunctionType.Exp)
                    nc.tensor.matmul(
                        ops[:], v_aug[:, kt, :], pt[:],
                        start=(kt == 0), stop=(kt == nkt - 1),
                    )
                osb = owork.tile([HD + 1, CH], F32, name="osb", tag="osb")
                nc.vector.tensor_copy(osb[:], ops[:])
                nc.sync.dma_start(out=outT[:, cs], in_=osb[:])

    if legalize:
        _legalize_waits(nc, mybir)
    return nc


def _legalize_waits(nc, mybir):
    """Split multi-wait instructions for the XLA-route walrus codegen.

    The TPB EVENTS struct holds one semaphore wait per instruction; this
    pipeline's codegen refuses >1 wait on generic structs (>2 on Matmult,
    whose LDWEIGHTS+MATMUL pair absorbs two). Hoist extra waits onto
    standalone EventSemaphore instructions on the same engine queue right
    before the instruction - semantically identical, the queue stalls there.
    """
    n = 0
    for f in nc.m.functions:
        for b in f.blocks:
            out = []
            changed = False
            for inst in b.instructions:
                si = inst.sync_info
                waits = list(si.on_wait) if si is not None and si.on_wait else []
                limit = 1
                if len(waits) > limit:
                    changed = True
                    for w in waits[:-limit]:
                        n += 1
                        out.append(mybir.InstEventSemaphore(
                            name=f"waitfix{n}_{inst.name}",
                            engine=inst.engine,
                            sync_info=mybir.SyncInfo(on_wait=[w], on_update=[]),
                        ))
                    inst.sync_info = mybir.SyncInfo(
                        on_wait=waits[-limit:],
                        on_update=list(si.on_update or []),
                    )
                out.append(inst)
            if changed:
                b.instructions = out
    return n


def kernel(x, wq, bq, wk, bk, wv, bv):
    global LAST_RESULTS
    import os
    os.environ.setdefault("JAX_PLATFORMS", "")
    from concourse.bass_utils import run_bass_kernel_spmd

    mmdt = {"float16": np.float16, "bfloat16": None, "float32": np.float32,
            "float32r": np.float32}[_MM_DTYPE]
    x = np.asarray(x, dtype=np.float32)
    s = np.float32(1.0 / np.sqrt(HD))
    wqT = np.ascontiguousarray((np.asarray(wq, np.float32) * s).T.astype(mmdt))
    wkT = np.ascontiguousarray(np.asarray(wk, np.float32).T.astype(mmdt))
    wvT = np.ascontiguousarray(np.asarray(wv, np.float32).T.astype(mmdt))
    bq_c = np.ascontiguousarray((np.asarray(bq, np.float32) * s).reshape(HD, 1))
    bk_c = np.ascontiguousarray(np.asarray(bk, np.float32).reshape(HD, 1))
    bv_c = np.ascontiguousarray(np.asarray(bv, np.float32).reshape(HD, 1))
    xT = np.ascontiguousarray(np.swapaxes(x, 1, 2).astype(mmdt))  # (B, D, T)

    nc = _build_module()
    in_maps = [
        {"xT": xT[b], "wqT": wqT, "wkT": wkT, "wvT": wvT,
         "bq": bq_c, "bk": bk_c, "bv": bv_c}
        for b in range(B)
    ]
    res = run_bass_kernel_spmd(nc, in_maps, core_ids=list(range(B)))
    LAST_RESULTS = res

    out = np.empty((B, T, HD), dtype=np.float32)
    for b in range(B):
        oT = res.results[b]["outT"]  # (65, T): rows 0..63 = O^T, row 64 = denom
        out[b] = (oT[:HD] / oT[HD:HD + 1]).T
    return out


# revision 11
# speedup vs baseline: 1.2131x; 1.2131x over previous
"""Trainium2 Bass kernel for a single causal attention head.

Problem: x:(8,2048,1024) f32, per-head projections wq/wk/wv:(64,1024),
biases (64,). Output: softmax(causal(q k^T / sqrt(64))) @ v : (8,2048,64).

Strategy:
  - Data-parallel: batch b -> core b (8 cores, 1 batch each).
  - Host prep: x[b] transposed to xT:(1024,2048) fp16 so the contraction dim
    (D) lands on SBUF partitions; Q/K weights shipped twice as stacked pairs
    [wq|wk] and [wk|wq] (fp16, 1/sqrt(64) folded into wq) so Q^T and K^T both
    exist on partitions 0-63 AND 64-127 - enabling row-packed score matmuls.
  - Device (per core):
      * qk1/qk2 = stackT.T @ xT (PSUM accumulate over 8 d-tiles, fp16, N=512)
      * vT transposed back to (T,64) tiles via PE transpose, augmented with a
        ones column (-> softmax denominator rides along the PV matmul).
      * scores transposed S^T[j,i] = sum_h K^T[h,j] Q^T[h,i], TWO k-tiles per
        PE window via row packing (contraction=64: rows 0-63 & 64-127).
      * P^T = exp(S^T) on ACT, one [128,1024] instr per k-tile pair; causal
        masking = GPSIMD affine_select zeroing P^T above the diagonal
        (identical result: zeros add nothing to numerator or denominator).
      * O^T_aug[65, T] accumulated in PSUM over k-tiles; row 64 = sum_j P^T.
      * causal skip: k-tiles entirely above the diagonal never computed.
  - Host post: out[b] = (O^T[0:64] / O^T[64:65]).T  (softmax normalization).
"""

import numpy as np

B, T, D, HD = 8, 2048, 1024, 64
P = 128          # SBUF partitions
CH = 512         # q-chunk (matmul moving dim)
NCH = T // CH    # 4
DT = D // P      # 8 d-tiles
NKT = T // P     # 16 k-tiles

LAST_RESULTS = None      # BassKernelResults of the most recent run (for test.py)


def _build_module(legalize=True):
    import concourse.bass as bass
    import concourse.mybir as mybir
    from concourse.tile import TileContext
    from concourse.masks import make_identity

    F32 = mybir.dt.float32
    F16 = mybir.dt.float16

    nc = bass.Bass("TRN2", target_bir_lowering=True)

    xT = nc.dram_tensor("xT", (D, T), F16, kind="ExternalInput")
    w1 = nc.dram_tensor("w1", (D, P), F16, kind="ExternalInput")   # [wq*s | wk]^T
    w2 = nc.dram_tensor("w2", (D, P), F16, kind="ExternalInput")   # [wk | wq*s]^T
    wv = nc.dram_tensor("wv", (D, HD), F16, kind="ExternalInput")  # wv^T
    b1 = nc.dram_tensor("b1", (P, 1), F32, kind="ExternalInput")   # [bq*s; bk]
    b2 = nc.dram_tensor("b2", (P, 1), F32, kind="ExternalInput")   # [bk; bq*s]
    bv = nc.dram_tensor("bv", (HD, 1), F32, kind="ExternalInput")
    outT = nc.dram_tensor("outT", (HD + 1, T), F32, kind="ExternalOutput")

    with TileContext(nc) as tc:
        with (
            tc.tile_pool(name="const", bufs=1) as const,
            tc.tile_pool(name="acts", bufs=1) as acts,
            tc.tile_pool(name="proj_ps", bufs=2, space="PSUM") as proj_ps,
            tc.tile_pool(name="tr_ps", bufs=1, space="PSUM") as tr_ps,
            tc.tile_pool(name="s_ps", bufs=2, space="PSUM") as s_ps,
            tc.tile_pool(name="o_ps", bufs=1, space="PSUM") as o_ps,
            tc.tile_pool(name="pwork", bufs=4) as pwork,
            tc.tile_pool(name="owork", bufs=2) as owork,
        ):
            # ---- constants / weights ----
            ident = const.tile([P, P], F32, name="ident")
            make_identity(nc, ident)

            w_sb = {}
            for nm, dram, m in (("w1", w1, P), ("w2", w2, P), ("wv", wv, HD)):
                w = const.tile([P, DT, m], F16, name=f"{nm}_sb")
                nc.sync.dma_start(out=w[:], in_=dram.rearrange("(n p) h -> p n h", p=P))
                w_sb[nm] = w
            b_sb = {}
            for nm, dram, m in (("b1", b1, P), ("b2", b2, P), ("bv", bv, HD)):
                b = const.tile([m, 1], F32, name=f"{nm}_sb")
                nc.sync.dma_start(out=b[:], in_=dram[:, :])
                b_sb[nm] = b

            xt = []
            for d in range(DT):
                t = const.tile([P, T], F16, name=f"xt{d}")
                nc.sync.dma_start(out=t[:], in_=xT[d * P:(d + 1) * P, :])
                xt.append(t)

            # ---- activations ----
            # qk1: rows 0-63 = Q^T, rows 64-127 = K^T; qk2: swapped.
            qk1 = acts.tile([P, T], F16, name="qk1")
            qk2 = acts.tile([P, T], F16, name="qk2")
            vT = acts.tile([HD, T], F32, name="vT")
            v_aug = acts.tile([P, NKT, HD + 1], F16, name="v_aug")
            nc.vector.memset(v_aug[:, :, HD], 1.0)

            # ---- projections ----
            for ci in range(NCH):
                cs = slice(ci * CH, (ci + 1) * CH)
                for nm, bnm, dst, m in (("w1", "b1", qk1, P), ("w2", "b2", qk2, P),
                                        ("wv", "bv", vT, HD)):
                    ps = proj_ps.tile([m, CH], F32, name="proj", tag="proj")
                    for d in range(DT):
                        nc.tensor.matmul(
                            ps[:], w_sb[nm][:, d, :], xt[d][:, cs],
                            start=(d == 0), stop=(d == DT - 1),
                        )
                    nc.vector.tensor_scalar_add(dst[:, cs], ps[:], b_sb[bnm][:])
                # vT chunk -> v_aug tiles (T on partitions) as soon as ready
                for tt in range(4 * ci, 4 * ci + 4):
                    tp = tr_ps.tile([P, HD], F32, name="vtr", tag="vtr")
                    nc.tensor.transpose(tp[:], vT[:, tt * P:(tt + 1) * P], ident[:HD, :HD])
                    nc.vector.tensor_copy(v_aug[:, tt, 0:HD], tp[:])

            # ---- attention, q-chunk at a time, k-tiles in row-packed pairs ----
            for ci in range(NCH):
                cs = slice(ci * CH, (ci + 1) * CH)
                nkt = 4 * (ci + 1)
                ops = o_ps.tile([HD + 1, CH], F32, name="oacc", tag="oacc")
                for j in range(nkt // 2):
                    ka, kb = 2 * j, 2 * j + 1
                    s2 = s_ps.tile([P, 2 * CH], F32, name="sT", tag="sT")
                    # rows 0-63 of the array: K^T from qk2, Q^T from qk1
                    nc.tensor.matmul(
                        s2[:, 0:CH], qk2[0:HD, ka * P:(ka + 1) * P],
                        qk1[0:HD, cs], start=True, stop=True,
                    )
                    # rows 64-127: K^T from qk1, Q^T from qk2 (concurrent)
                    nc.tensor.matmul(
                        s2[:, CH:2 * CH], qk1[HD:P, kb * P:(kb + 1) * P],
                        qk2[HD:P, cs], start=True, stop=True,
                    )
                    pt = pwork.tile([P, 2 * CH], F16, name="pT", tag="pT")
                    nc.scalar.activation(pt[:], s2[:], mybir.ActivationFunctionType.Exp)
                    # causal mask: zero P^T where key > query (diagonal band only)
                    for half, kt in ((0, ka), (1, kb)):
                        delta = kt * P - ci * CH
                        if delta >= 0:
                            nc.gpsimd.affine_select(
                                out=pt[:, half * CH:(half + 1) * CH],
                                in_=pt[:, half * CH:(half + 1) * CH],
                                compare_op=mybir.AluOpType.is_ge, fill=0.0,
                                base=-delta, pattern=[[1, CH]], channel_multiplier=-1,
                            )
                    nc.tensor.matmul(
                        ops[:], v_aug[:, ka, :], pt[:, 0:CH],
                        start=(j == 0), stop=False,
                    )
                    nc.tensor.matmul(
                        ops[:], v_aug[:, kb, :], pt[:, CH:2 * CH],
                        start=False, stop=(j == nkt // 2 - 1),
                    )
                osb = owork.tile([HD + 1, CH], F32, name="osb", tag="osb")
                nc.vector.tensor_copy(osb[:], ops[:])
                nc.sync.dma_start(out=outT[:, cs], in_=osb[:])

    if legalize:
        _legalize_waits(nc, mybir)
    return nc


def _legalize_waits(nc, mybir):
    """Split multi-wait instructions for the XLA-route walrus codegen.

    The TPB EVENTS struct holds one semaphore wait per instruction and this
    pipeline's codegen refuses >1. Hoist extra waits onto standalone
    EventSemaphore instructions on the same engine queue right before the
    instruction - semantically identical, the queue stalls there.
    """
    n = 0
    for f in nc.m.functions:
        for b in f.blocks:
            out = []
            changed = False
            for inst in b.instructions:
                si = inst.sync_info
                waits = list(si.on_wait) if si is not None and si.on_wait else []
                limit = 1
                if len(waits) > limit:
                    changed = True
                    for w in waits[:-limit]:
                        n += 1
                        out.append(mybir.InstEventSemaphore(
                            name=f"waitfix{n}_{inst.name}",
                            engine=inst.engine,
                            sync_info=mybir.SyncInfo(on_wait=[w], on_update=[]),
                        ))
                    inst.sync_info = mybir.SyncInfo(
                        on_wait=waits[-limit:],
                        on_update=list(si.on_update or []),
                    )
                out.append(inst)
            if changed:
                b.instructions = out
    return n


def kernel(x, wq, bq, wk, bk, wv, bv):
    global LAST_RESULTS
    import os
    os.environ.setdefault("JAX_PLATFORMS", "")
    from concourse.bass_utils import run_bass_kernel_spmd

    x = np.asarray(x, dtype=np.float32)
    s = np.float32(1.0 / np.sqrt(HD))
    wq_s = np.asarray(wq, np.float32) * s
    wk_f = np.asarray(wk, np.float32)
    w1 = np.ascontiguousarray(np.concatenate([wq_s, wk_f], 0).T.astype(np.float16))
    w2 = np.ascontiguousarray(np.concatenate([wk_f, wq_s], 0).T.astype(np.float16))
    wv_c = np.ascontiguousarray(np.asarray(wv, np.float32).T.astype(np.float16))
    bq_s = np.asarray(bq, np.float32) * s
    bk_f = np.asarray(bk, np.float32)
    b1 = np.ascontiguousarray(np.concatenate([bq_s, bk_f]).reshape(P, 1))
    b2 = np.ascontiguousarray(np.concatenate([bk_f, bq_s]).reshape(P, 1))
    bv_c = np.ascontiguousarray(np.asarray(bv, np.float32).reshape(HD, 1))
    xT = np.ascontiguousarray(np.swapaxes(x, 1, 2).astype(np.float16))  # (B, D, T)

    nc = _build_module()
    in_maps = [
        {"xT": xT[b], "w1": w1, "w2": w2, "wv": wv_c,
         "b1": b1, "b2": b2, "bv": bv_c}
        for b in range(B)
    ]
    res = run_bass_kernel_spmd(nc, in_maps, core_ids=list(range(B)))
    LAST_RESULTS = res

    out = np.empty((B, T, HD), dtype=np.float32)
    for b in range(B):
        oT = res.results[b]["outT"]  # (65, T): rows 0..63 = O^T, row 64 = denom
        out[b] = (oT[:HD] / oT[HD:HD + 1]).T
    return out


# revision 14
# speedup vs baseline: 1.4720x; 1.2135x over previous
"""Trainium2 Bass kernel for a single causal attention head.

Problem: x:(8,2048,1024) f32, per-head projections wq/wk/wv:(64,1024),
biases (64,). Output: softmax(causal(q k^T / sqrt(64))) @ v : (8,2048,64).

Strategy:
  - Data-parallel: batch b -> core b (8 cores, 1 batch each).
  - Host prep: x[b] transposed to xT:(1024,2048) fp16 (contraction dim D on
    SBUF partitions); Q/K weights shipped stacked as [wq|wk] (fp16, 1/sqrt(64)
    folded into wq).
  - Device (per core):
      * qk1 = [wq|wk]^T.T @ xT: rows 0-63 = Q^T, rows 64-127 = K^T (PSUM
        accumulate over 8 d-tiles, fp16 matmuls, N=512 chunks).
      * qk2 = half-swapped copy of qk1 (SBUF->SBUF DMA): K^T on rows 0-63,
        Q^T on rows 64-127. Both copies exist on both partition halves ->
        scores for TWO k-tiles run concurrently via PE row packing.
      * vT transposed back to (T,64) tiles via PE transpose, augmented with a
        ones column (softmax denominator rides along the PV matmul).
      * S^T[j,i] = sum_h K^T[h,j] Q^T[h,i] transposed-scores layout; P^T =
        exp(S^T) on ACT, one [128,1024] instr per k-tile pair; causal mask =
        GPSIMD affine_select zeroing P^T above the diagonal (identical
        result: zeros add nothing to numerator or denominator).
      * O^T_aug[65, T] accumulated in PSUM over k-tiles; row 64 = sum_j P^T.
      * causal skip: k-tiles entirely above the diagonal never computed.
      * attention for chunk ci is emitted right after projection chunk ci, so
        exp/PV overlap later projections instead of serializing at the end.
  - Host post: out[b] = (O^T[0:64] / O^T[64:65]).T  (softmax normalization).
"""

import numpy as np

B, T, D, HD = 8, 2048, 1024, 64
P = 128          # SBUF partitions
CH = 512         # q-chunk (matmul moving dim)
NCH = T // CH    # 4
DT = D // P      # 8 d-tiles
NKT = T // P     # 16 k-tiles
HT = T // 2      # xT half-tile width

LAST_RESULTS = None      # BassKernelResults of the most recent run (for test.py)


def _build_module(legalize=True):
    import concourse.bass as bass
    import concourse.mybir as mybir
    from concourse.tile import TileContext
    from concourse.masks import make_identity

    F32 = mybir.dt.float32
    F16 = mybir.dt.float16

    nc = bass.Bass("TRN2", target_bir_lowering=True)

    xT = nc.dram_tensor("xT", (D, T), F16, kind="ExternalInput")
    w1 = nc.dram_tensor("w1", (D, P), F16, kind="ExternalInput")   # [wq*s | wk]^T
    wv = nc.dram_tensor("wv", (D, HD), F16, kind="ExternalInput")  # wv^T
    b1 = nc.dram_tensor("b1", (P, 1), F32, kind="ExternalInput")   # [bq*s; bk]
    bv = nc.dram_tensor("bv", (HD, 1), F32, kind="ExternalInput")
    outT = nc.dram_tensor("outT", (HD + 1, T), F32, kind="ExternalOutput")

    with TileContext(nc) as tc:
        with (
            tc.tile_pool(name="const", bufs=1) as const,
            tc.tile_pool(name="acts", bufs=1) as acts,
            tc.tile_pool(name="proj_ps", bufs=2, space="PSUM") as proj_ps,
            tc.tile_pool(name="tr_ps", bufs=1, space="PSUM") as tr_ps,
            tc.tile_pool(name="s_ps", bufs=2, space="PSUM") as s_ps,
            tc.tile_pool(name="o_ps", bufs=1, space="PSUM") as o_ps,
            tc.tile_pool(name="pwork", bufs=4) as pwork,
            tc.tile_pool(name="owork", bufs=2) as owork,
        ):
            # ---- weights / biases first (small), then x half-tiles in
            # consumption order so chunk-0 projections start early ----
            w1_sb = const.tile([P, DT, P], F16, name="w1_sb")
            nc.sync.dma_start(out=w1_sb[:], in_=w1.rearrange("(n p) h -> p n h", p=P))
            wv_sb = const.tile([P, DT, HD], F16, name="wv_sb")
            nc.sync.dma_start(out=wv_sb[:], in_=wv.rearrange("(n p) h -> p n h", p=P))
            b1_sb = const.tile([P, 1], F32, name="b1_sb")
            nc.sync.dma_start(out=b1_sb[:], in_=b1[:, :])
            bv_sb = const.tile([HD, 1], F32, name="bv_sb")
            nc.sync.dma_start(out=bv_sb[:], in_=bv[:, :])

            ident = const.tile([P, P], F32, name="ident")
            make_identity(nc, ident)

            xt = [[None] * DT, [None] * DT]
            for h in range(2):
                for d in range(DT):
                    t = const.tile([P, HT], F16, name=f"xt{h}_{d}")
                    nc.sync.dma_start(
                        out=t[:], in_=xT[d * P:(d + 1) * P, h * HT:(h + 1) * HT])
                    xt[h][d] = t

            # ---- activations ----
            # qk1: rows 0-63 = Q^T, rows 64-127 = K^T; qk2: swapped halves.
            qk1 = acts.tile([P, T], F16, name="qk1")
            qk2 = acts.tile([P, T], F16, name="qk2")
            vT = acts.tile([HD, T], F32, name="vT")
            v_aug = acts.tile([P, NKT, HD + 1], F16, name="v_aug")
            nc.vector.memset(v_aug[:, :, HD], 1.0)

            def proj_chunk(ci):
                cs = slice(ci * CH, (ci + 1) * CH)
                rhs = xt[ci // 2]
                rs = slice((ci % 2) * CH, (ci % 2) * CH + CH)
                ps = proj_ps.tile([P, CH], F32, name="proj", tag="proj")
                for d in range(DT):
                    nc.tensor.matmul(ps[:], w1_sb[:, d, :], rhs[d][:, rs],
                                     start=(d == 0), stop=(d == DT - 1))
                nc.vector.tensor_scalar_add(qk1[:, cs], ps[:], b1_sb[:])
                # half-swapped copy: qk2 = [K^T; Q^T]
                nc.sync.dma_start(out=qk2[0:HD, cs], in_=qk1[HD:P, cs])
                nc.sync.dma_start(out=qk2[HD:P, cs], in_=qk1[0:HD, cs])
                psv = proj_ps.tile([HD, CH], F32, name="projv", tag="proj",
                                   padded_shape=[P, CH])
                for d in range(DT):
                    nc.tensor.matmul(psv[:], wv_sb[:, d, :], rhs[d][:, rs],
                                     start=(d == 0), stop=(d == DT - 1))
                nc.vector.tensor_scalar_add(vT[:, cs], psv[:], bv_sb[:])
                for tt in range(4 * ci, 4 * ci + 4):
                    tp = tr_ps.tile([P, HD], F32, name="vtr", tag="vtr")
                    nc.tensor.transpose(tp[:], vT[:, tt * P:(tt + 1) * P],
                                        ident[:HD, :HD])
                    nc.vector.tensor_copy(v_aug[:, tt, 0:HD], tp[:])

            def attn_chunk(ci):
                cs = slice(ci * CH, (ci + 1) * CH)
                nkt = 4 * (ci + 1)
                ops = o_ps.tile([HD + 1, CH], F32, name="oacc", tag="oacc")
                for j in range(nkt // 2):
                    ka, kb = 2 * j, 2 * j + 1
                    s2 = s_ps.tile([P, 2 * CH], F32, name="sT", tag="sT")
                    # rows 0-63 of the array: K^T from qk2, Q^T from qk1
                    nc.tensor.matmul(s2[:, 0:CH], qk2[0:HD, ka * P:(ka + 1) * P],
                                     qk1[0:HD, cs], start=True, stop=True)
                    # rows 64-127: K^T from qk1, Q^T from qk2 (concurrent)
                    nc.tensor.matmul(s2[:, CH:2 * CH], qk1[HD:P, kb * P:(kb + 1) * P],
                                     qk2[HD:P, cs], start=True, stop=True)
                    pt = pwork.tile([P, 2 * CH], F16, name="pT", tag="pT")
                    nc.scalar.activation(pt[:], s2[:],
                                         mybir.ActivationFunctionType.Exp)
                    # causal mask: zero P^T where key > query (diagonal band)
                    for half, kt in ((0, ka), (1, kb)):
                        delta = kt * P - ci * CH
                        if delta >= 0:
                            nc.gpsimd.affine_select(
                                out=pt[:, half * CH:(half + 1) * CH],
                                in_=pt[:, half * CH:(half + 1) * CH],
                                compare_op=mybir.AluOpType.is_ge, fill=0.0,
                                base=-delta, pattern=[[1, CH]],
                                channel_multiplier=-1,
                            )
                    nc.tensor.matmul(ops[:], v_aug[:, ka, :], pt[:, 0:CH],
                                     start=(j == 0), stop=False)
                    nc.tensor.matmul(ops[:], v_aug[:, kb, :], pt[:, CH:2 * CH],
                                     start=False, stop=(j == nkt // 2 - 1))
                osb = owork.tile([HD + 1, CH], F32, name="osb", tag="osb")
                nc.vector.tensor_copy(osb[:], ops[:])
                nc.sync.dma_start(out=outT[:, cs], in_=osb[:])

            for ci in range(NCH):
                proj_chunk(ci)
                attn_chunk(ci)

    if legalize:
        _legalize_waits(nc, mybir)
    return nc


def _legalize_waits(nc, mybir):
    """Split multi-wait instructions for the XLA-route walrus codegen.

    The TPB EVENTS struct holds one semaphore wait per instruction and this
    pipeline's codegen refuses >1. Hoist extra waits onto standalone
    EventSemaphore instructions on the same engine queue right before the
    instruction - semantically identical, the queue stalls there.
    """
    n = 0
    for f in nc.m.functions:
        for b in f.blocks:
            out = []
            changed = False
            for inst in b.instructions:
                si = inst.sync_info
                waits = list(si.on_wait) if si is not None and si.on_wait else []
                if len(waits) > 1:
                    changed = True
                    for w in waits[:-1]:
                        n += 1
                        out.append(mybir.InstEventSemaphore(
                            name=f"waitfix{n}_{inst.name}",
                            engine=inst.engine,
                            sync_info=mybir.SyncInfo(on_wait=[w], on_update=[]),
                        ))
                    inst.sync_info = mybir.SyncInfo(
                        on_wait=waits[-1:],
                        on_update=list(si.on_update or []),
                    )
                out.append(inst)
            if changed:
                b.instructions = out
    return n


def kernel(x, wq, bq, wk, bk, wv, bv):
    global LAST_RESULTS
    import os
    os.environ.setdefault("JAX_PLATFORMS", "")
    from concourse.bass_utils import run_bass_kernel_spmd

    x = np.asarray(x, dtype=np.float32)
    s = np.float32(1.0 / np.sqrt(HD))
    wq_s = np.asarray(wq, np.float32) * s
    wk_f = np.asarray(wk, np.float32)
    w1 = np.ascontiguousarray(np.concatenate([wq_s, wk_f], 0).T.astype(np.float16))
    wv_c = np.ascontiguousarray(np.asarray(wv, np.float32).T.astype(np.float16))
    b1 = np.ascontiguousarray(
        np.concatenate([np.asarray(bq, np.float32) * s,
                        np.asarray(bk, np.float32)]).reshape(P, 1))
    bv_c = np.ascontiguousarray(np.asarray(bv, np.float32).reshape(HD, 1))
    xT = np.ascontiguousarray(np.swapaxes(x, 1, 2).astype(np.float16))  # (B, D, T)

    nc = _build_module()
    in_maps = [
        {"xT": xT[b], "w1": w1, "wv": wv_c, "b1": b1, "bv": bv_c}
        for b in range(B)
    ]
    res = run_bass_kernel_spmd(nc, in_maps, core_ids=list(range(B)))
    LAST_RESULTS = res

    out = np.empty((B, T, HD), dtype=np.float32)
    for b in range(B):
        oT = res.results[b]["outT"]  # (65, T): rows 0..63 = O^T, row 64 = denom
        out[b] = (oT[:HD] / oT[HD:HD + 1]).T
    return out


# revision 20
# speedup vs baseline: 1.4927x; 1.0140x over previous
"""Trainium2 Bass kernel for a single causal attention head.

Problem: x:(8,2048,1024) f32, per-head projections wq/wk/wv:(64,1024),
biases (64,). Output: softmax(causal(q k^T / sqrt(64))) @ v : (8,2048,64).

Strategy:
  - Data-parallel: batch b -> core b (8 cores, 1 batch each).
  - Host prep: x[b] transposed to xT:(1024,2048) fp16 (contraction dim D on
    SBUF partitions); Q/K weights shipped stacked as [wq|wk] (fp16, 1/sqrt(64)
    folded into wq).
  - Device (per core):
      * qk1 = [wq|wk]^T.T @ xT: rows 0-63 = Q^T, rows 64-127 = K^T (PSUM
        accumulate over 8 d-tiles, fp16 matmuls, N=512 chunks).
      * qk2 = half-swapped copy of qk1 (SBUF->SBUF DMA): K^T on rows 0-63,
        Q^T on rows 64-127. Both copies exist on both partition halves ->
        scores for TWO k-tiles run concurrently via PE row packing.
      * vT transposed back to (T,64) tiles via PE transpose, augmented with a
        ones column (softmax denominator rides along the PV matmul).
      * S^T[j,i] = sum_h K^T[h,j] Q^T[h,i] transposed-scores layout; P^T =
        exp(S^T) on ACT, one [128,1024] instr per k-tile pair; causal mask =
        GPSIMD affine_select zeroing P^T above the diagonal (identical
        result: zeros add nothing to numerator or denominator).
      * O^T_aug[65, T] accumulated in PSUM over k-tiles; row 64 = sum_j P^T.
      * causal skip: k-tiles entirely above the diagonal never computed.
      * attention for chunk ci is emitted right after projection chunk ci, so
        exp/PV overlap later projections instead of serializing at the end.
  - Host post: out[b] = (O^T[0:64] / O^T[64:65]).T  (softmax normalization).
"""

import numpy as np

B, T, D, HD = 8, 2048, 1024, 64
P = 128          # SBUF partitions
CH = 512         # q-chunk (matmul moving dim)
NCH = T // CH    # 4
DT = D // P      # 8 d-tiles
NKT = T // P     # 16 k-tiles
HT = T // 2      # xT half-tile width

LAST_RESULTS = None      # BassKernelResults of the most recent run (for test.py)


def _build_module(legalize=True):
    import concourse.bass as bass
    import concourse.mybir as mybir
    from concourse.tile import TileContext

    F32 = mybir.dt.float32
    F16 = mybir.dt.float16

    nc = bass.Bass("TRN2", target_bir_lowering=True)

    xT = nc.dram_tensor("xT", (D, T), F16, kind="ExternalInput")
    w1 = nc.dram_tensor("w1", (D, P), F16, kind="ExternalInput")   # [wq*s | wk]^T
    wv = nc.dram_tensor("wv", (D, HD), F16, kind="ExternalInput")  # wv^T
    b1 = nc.dram_tensor("b1", (P, 1), F32, kind="ExternalInput")   # [bq*s; bk]
    bv = nc.dram_tensor("bv", (HD, 1), F32, kind="ExternalInput")
    outT = nc.dram_tensor("outT", (HD + 1, T), F32, kind="ExternalOutput")

    with TileContext(nc) as tc:
        with (
            tc.tile_pool(name="const", bufs=1) as const,
            tc.tile_pool(name="acts", bufs=1) as acts,
            tc.tile_pool(name="proj_ps", bufs=2, space="PSUM") as proj_ps,
            tc.tile_pool(name="s_ps", bufs=2, space="PSUM") as s_ps,
            tc.tile_pool(name="o_ps", bufs=2, space="PSUM") as o_ps,
            tc.tile_pool(name="pwork", bufs=4) as pwork,
            tc.tile_pool(name="owork", bufs=2) as owork,
        ):
            # ---- weights / biases first (small), then x half-tiles in
            # consumption order so chunk-0 projections start early ----
            w1_sb = const.tile([P, DT, P], F16, name="w1_sb")
            nc.sync.dma_start(out=w1_sb[:], in_=w1.rearrange("(n p) h -> p n h", p=P))

            # xT as 4 x 1MB loads (d-tiles 0-3 / 4-7 per T-half), interleaved
            # with the small weight/bias loads in consumption order
            xr = xT.rearrange("(n p) t -> p n t", p=P)
            xt = [[None, None], [None, None]]
            hs0 = slice(0, HT)
            xt[0][0] = const.tile([P, 4, HT], F16, name="xt0a")
            nc.sync.dma_start(out=xt[0][0][:], in_=xr[:, 0:4, hs0])
            wv_sb = const.tile([P, DT, HD], F16, name="wv_sb")
            nc.sync.dma_start(out=wv_sb[:], in_=wv.rearrange("(n p) h -> p n h", p=P))
            xt[0][1] = const.tile([P, 4, HT], F16, name="xt0b")
            nc.sync.dma_start(out=xt[0][1][:], in_=xr[:, 4:8, hs0])
            b1_sb = const.tile([P, 1], F32, name="b1_sb")
            nc.sync.dma_start(out=b1_sb[:], in_=b1[:, :])
            bv_sb = const.tile([HD, 1], F32, name="bv_sb")
            nc.sync.dma_start(out=bv_sb[:], in_=bv[:, :])
            hs1 = slice(HT, T)
            xt[1][0] = const.tile([P, 4, HT], F16, name="xt1a")
            nc.sync.dma_start(out=xt[1][0][:], in_=xr[:, 0:4, hs1])
            xt[1][1] = const.tile([P, 4, HT], F16, name="xt1b")
            nc.sync.dma_start(out=xt[1][1][:], in_=xr[:, 4:8, hs1])

            # ---- activations ----
            # qk1: rows 0-63 = Q^T, rows 64-127 = K^T; qk2: swapped halves.
            qk1 = acts.tile([P, T], F16, name="qk1")
            qk2 = acts.tile([P, T], F16, name="qk2")
            vT = acts.tile([HD, T], F16, name="vT")
            v_aug = acts.tile([P, NKT, 80], F16, name="v_aug")  # 80: 32B-aligned rows for xbar transpose
            nc.vector.memset(v_aug[:, :, HD], 1.0)

            def proj_chunk(ci):
                cs = slice(ci * CH, (ci + 1) * CH)
                rhs = xt[ci // 2]
                rs = slice((ci % 2) * CH, (ci % 2) * CH + CH)
                ps = proj_ps.tile([P, CH], F32, name="proj", tag="proj")
                for d in range(DT):
                    nc.tensor.matmul(ps[:], w1_sb[:, d, :], rhs[d // 4][:, d % 4, rs],
                                     start=(d == 0), stop=(d == DT - 1))
                nc.vector.tensor_scalar_add(qk1[:, cs], ps[:], b1_sb[:])
                # half-swapped copy: qk2 = [K^T; Q^T]
                nc.sync.dma_start(out=qk2[0:HD, cs], in_=qk1[HD:P, cs])
                nc.sync.dma_start(out=qk2[HD:P, cs], in_=qk1[0:HD, cs])
                psv = proj_ps.tile([HD, CH], F32, name="projv", tag="proj",
                                   padded_shape=[P, CH])
                for d in range(DT):
                    nc.tensor.matmul(psv[:], wv_sb[:, d, :], rhs[d // 4][:, d % 4, rs],
                                     start=(d == 0), stop=(d == DT - 1))
                nc.vector.tensor_scalar_add(vT[:, cs], psv[:], bv_sb[:])
                for tt in range(4 * ci, 4 * ci + 4):
                    nc.sync.dma_start_transpose(out=v_aug[:, tt, 0:HD],
                                                in_=vT[:, tt * P:(tt + 1) * P])

            def attn_chunk(ci):
                cs = slice(ci * CH, (ci + 1) * CH)
                nkt = 4 * (ci + 1)
                ops = o_ps.tile([HD + 1, CH], F32, name="oacc", tag="oacc")
                for j in range(nkt // 2):
                    ka, kb = 2 * j, 2 * j + 1
                    s2 = s_ps.tile([P, 2 * CH], F32, name="sT", tag="sT")
                    # rows 0-63 of the array: K^T from qk2, Q^T from qk1
                    nc.tensor.matmul(s2[:, 0:CH], qk2[0:HD, ka * P:(ka + 1) * P],
                                     qk1[0:HD, cs], start=True, stop=True)
                    # rows 64-127: K^T from qk1, Q^T from qk2 (concurrent)
                    nc.tensor.matmul(s2[:, CH:2 * CH], qk1[HD:P, kb * P:(kb + 1) * P],
                                     qk2[HD:P, cs], start=True, stop=True)
                    pt = pwork.tile([P, 2 * CH], F16, name="pT", tag="pT")
                    nc.scalar.activation(pt[:], s2[:],
                                         mybir.ActivationFunctionType.Exp)
                    # causal mask: zero P^T where key > query (diagonal band)
                    for half, kt in ((0, ka), (1, kb)):
                        delta = kt * P - ci * CH
                        if delta >= 0:
                            nc.gpsimd.affine_select(
                                out=pt[:, half * CH:(half + 1) * CH],
                                in_=pt[:, half * CH:(half + 1) * CH],
                                compare_op=mybir.AluOpType.is_ge, fill=0.0,
                                base=-delta, pattern=[[1, CH]],
                                channel_multiplier=-1,
                            )
                    nc.tensor.matmul(ops[:], v_aug[:, ka, 0:HD + 1], pt[:, 0:CH],
                                     start=(j == 0), stop=False)
                    nc.tensor.matmul(ops[:], v_aug[:, kb, 0:HD + 1], pt[:, CH:2 * CH],
                                     start=False, stop=(j == nkt // 2 - 1))
                osb = owork.tile([HD + 1, CH], F32, name="osb", tag="osb")
                nc.vector.tensor_copy(osb[:], ops[:])
                nc.sync.dma_start(out=outT[:, cs], in_=osb[:])

            for ci in range(NCH):
                proj_chunk(ci)
                attn_chunk(ci)

    if legalize:
        _legalize_waits(nc, mybir)
    return nc


def _legalize_waits(nc, mybir):
    """Split multi-wait instructions for the XLA-route walrus codegen.

    The TPB EVENTS struct holds one semaphore wait per instruction and this
    pipeline's codegen refuses >1. Hoist extra waits onto standalone
    EventSemaphore instructions on the same engine queue right before the
    instruction - semantically identical, the queue stalls there.
    """
    n = 0
    for f in nc.m.functions:
        for b in f.blocks:
            out = []
            changed = False
            for inst in b.instructions:
                si = inst.sync_info
                waits = list(si.on_wait) if si is not None and si.on_wait else []
                if len(waits) > 1:
                    changed = True
                    for w in waits[:-1]:
                        n += 1
                        out.append(mybir.InstEventSemaphore(
                            name=f"waitfix{n}_{inst.name}",
                            engine=inst.engine,
                            sync_info=mybir.SyncInfo(on_wait=[w], on_update=[]),
                        ))
                    inst.sync_info = mybir.SyncInfo(
                        on_wait=waits[-1:],
                        on_update=list(si.on_update or []),
                    )
                out.append(inst)
            if changed:
                b.instructions = out
    return n


def kernel(x, wq, bq, wk, bk, wv, bv):
    global LAST_RESULTS
    import os
    os.environ.setdefault("JAX_PLATFORMS", "")
    from concourse.bass_utils import run_bass_kernel_spmd

    x = np.asarray(x, dtype=np.float32)
    s = np.float32(1.0 / np.sqrt(HD))
    wq_s = np.asarray(wq, np.float32) * s
    wk_f = np.asarray(wk, np.float32)
    w1 = np.ascontiguousarray(np.concatenate([wq_s, wk_f], 0).T.astype(np.float16))
    wv_c = np.ascontiguousarray(np.asarray(wv, np.float32).T.astype(np.float16))
    b1 = np.ascontiguousarray(
        np.concatenate([np.asarray(bq, np.float32) * s,
                        np.asarray(bk, np.float32)]).reshape(P, 1))
    bv_c = np.ascontiguousarray(np.asarray(bv, np.float32).reshape(HD, 1))
    xT = np.ascontiguousarray(np.swapaxes(x, 1, 2).astype(np.float16))  # (B, D, T)

    nc = _build_module()
    in_maps = [
        {"xT": xT[b], "w1": w1, "wv": wv_c, "b1": b1, "bv": bv_c}
        for b in range(B)
    ]
    res = run_bass_kernel_spmd(nc, in_maps, core_ids=list(range(B)))
    LAST_RESULTS = res

    out = np.empty((B, T, HD), dtype=np.float32)
    for b in range(B):
        oT = res.results[b]["outT"]  # (65, T): rows 0..63 = O^T, row 64 = denom
        out[b] = (oT[:HD] / oT[HD:HD + 1]).T
    return out


# revision 21
# speedup vs baseline: 1.5886x; 1.0643x over previous
"""Trainium2 Bass kernel for a single causal attention head.

Problem: x:(8,2048,1024) f32, per-head projections wq/wk/wv:(64,1024),
biases (64,). Output: softmax(causal(q k^T / sqrt(64))) @ v : (8,2048,64).

Strategy:
  - Data-parallel: batch b -> core b (8 cores, 1 batch each).
  - Host prep: x[b] transposed to xT:(1024,2048) fp16 (contraction dim D on
    SBUF partitions); Q/K weights shipped stacked as [wq|wk] (fp16, 1/sqrt(64)
    folded into wq).
  - Device (per core):
      * qk1 = [wq|wk]^T.T @ xT: rows 0-63 = Q^T, rows 64-127 = K^T (PSUM
        accumulate over 8 d-tiles, fp16 matmuls, N=512 chunks).
      * qk2 = half-swapped copy of qk1 (SBUF->SBUF DMA): K^T on rows 0-63,
        Q^T on rows 64-127. Both copies exist on both partition halves ->
        scores for TWO k-tiles run concurrently via PE row packing.
      * vT transposed back to (T,64) tiles via PE transpose, augmented with a
        ones column (softmax denominator rides along the PV matmul).
      * S^T[j,i] = sum_h K^T[h,j] Q^T[h,i] transposed-scores layout; P^T =
        exp(S^T) on ACT, one [128,1024] instr per k-tile pair; causal mask =
        GPSIMD affine_select zeroing P^T above the diagonal (identical
        result: zeros add nothing to numerator or denominator).
      * O^T_aug[65, T] accumulated in PSUM over k-tiles; row 64 = sum_j P^T.
      * causal skip: k-tiles entirely above the diagonal never computed.
      * attention for chunk ci is emitted right after projection chunk ci, so
        exp/PV overlap later projections instead of serializing at the end.
  - Host post: out[b] = (O^T[0:64] / O^T[64:65]).T  (softmax normalization).
"""

import numpy as np

B, T, D, HD = 8, 2048, 1024, 64
P = 128          # SBUF partitions
CH = 512         # q-chunk (matmul moving dim)
NCH = T // CH    # 4
DT = D // P      # 8 d-tiles
NKT = T // P     # 16 k-tiles
HT = T // 2      # xT half-tile width

LAST_RESULTS = None      # BassKernelResults of the most recent run (for test.py)


def _build_module(legalize=True):
    import concourse.bass as bass
    import concourse.mybir as mybir
    from concourse.tile import TileContext

    from concourse.masks import make_identity
    F32 = mybir.dt.float32
    F16 = mybir.dt.float16

    nc = bass.Bass("TRN2", target_bir_lowering=True)

    xT = nc.dram_tensor("xT", (D, T), F16, kind="ExternalInput")
    w1 = nc.dram_tensor("w1", (D, P), F16, kind="ExternalInput")   # [wq*s | wk]^T
    wv = nc.dram_tensor("wv", (D, HD), F16, kind="ExternalInput")  # wv^T
    b1 = nc.dram_tensor("b1", (P, 1), F32, kind="ExternalInput")   # [bq*s; bk]
    bv = nc.dram_tensor("bv", (HD, 1), F32, kind="ExternalInput")
    outT = nc.dram_tensor("outT", (HD + 1, T), F32, kind="ExternalOutput")

    with TileContext(nc) as tc:
        with (
            tc.tile_pool(name="const", bufs=1) as const,
            tc.tile_pool(name="acts", bufs=1) as acts,
            tc.tile_pool(name="proj_ps", bufs=2, space="PSUM") as proj_ps,
            tc.tile_pool(name="tr_ps", bufs=1, space="PSUM") as tr_ps,
            tc.tile_pool(name="s_ps", bufs=2, space="PSUM") as s_ps,
            tc.tile_pool(name="o_ps", bufs=1, space="PSUM") as o_ps,
            tc.tile_pool(name="pwork", bufs=4) as pwork,
            tc.tile_pool(name="owork", bufs=2) as owork,
        ):
            # ---- weights / biases first (small), then x half-tiles in
            # consumption order so chunk-0 projections start early ----
            ident = const.tile([P, P], F32, name="ident")
            make_identity(nc, ident)

            w1_sb = const.tile([P, DT, P], F16, name="w1_sb")
            nc.sync.dma_start(out=w1_sb[:], in_=w1.rearrange("(n p) h -> p n h", p=P))

            # xT as 4 x 1MB loads (d-tiles 0-3 / 4-7 per T-half), interleaved
            # with the small weight/bias loads in consumption order
            xr = xT.rearrange("(n p) t -> p n t", p=P)
            xt = [[None, None], [None, None]]
            hs0 = slice(0, HT)
            xt[0][0] = const.tile([P, 4, HT], F16, name="xt0a")
            nc.sync.dma_start(out=xt[0][0][:], in_=xr[:, 0:4, hs0])
            wv_sb = const.tile([P, DT, HD], F16, name="wv_sb")
            nc.sync.dma_start(out=wv_sb[:], in_=wv.rearrange("(n p) h -> p n h", p=P))
            xt[0][1] = const.tile([P, 4, HT], F16, name="xt0b")
            nc.sync.dma_start(out=xt[0][1][:], in_=xr[:, 4:8, hs0])
            b1_sb = const.tile([P, 1], F32, name="b1_sb")
            nc.sync.dma_start(out=b1_sb[:], in_=b1[:, :])
            bv_sb = const.tile([HD, 1], F32, name="bv_sb")
            nc.sync.dma_start(out=bv_sb[:], in_=bv[:, :])
            hs1 = slice(HT, T)
            xt[1][0] = const.tile([P, 4, HT], F16, name="xt1a")
            nc.sync.dma_start(out=xt[1][0][:], in_=xr[:, 0:4, hs1])
            xt[1][1] = const.tile([P, 4, HT], F16, name="xt1b")
            nc.sync.dma_start(out=xt[1][1][:], in_=xr[:, 4:8, hs1])

            # ---- activations ----
            # qk1: rows 0-63 = Q^T, rows 64-127 = K^T; qk2: swapped halves.
            qk1 = acts.tile([P, T], F16, name="qk1")
            qk2 = acts.tile([P, T], F16, name="qk2")
            vT = acts.tile([HD, T], F32, name="vT")
            v_aug = acts.tile([P, NKT, HD + 1], F16, name="v_aug")
            nc.vector.memset(v_aug[:, :, HD], 1.0)

            def proj_chunk(ci):
                cs = slice(ci * CH, (ci + 1) * CH)
                rhs = xt[ci // 2]
                rs = slice((ci % 2) * CH, (ci % 2) * CH + CH)
                ps = proj_ps.tile([P, CH], F32, name="proj", tag="proj")
                for d in range(DT):
                    nc.tensor.matmul(ps[:], w1_sb[:, d, :], rhs[d // 4][:, d % 4, rs],
                                     start=(d == 0), stop=(d == DT - 1))
                nc.vector.tensor_scalar_add(qk1[:, cs], ps[:], b1_sb[:])
                # half-swapped copy: qk2 = [K^T; Q^T]. 64-partition DVE ops
                # read any aligned src half and write either dest half.
                nc.vector.tensor_copy(qk2[0:HD, cs], qk1[HD:P, cs])
                nc.vector.tensor_copy(qk2[HD:P, cs], qk1[0:HD, cs])
                psv = proj_ps.tile([HD, CH], F32, name="projv", tag="proj",
                                   padded_shape=[P, CH])
                for d in range(DT):
                    nc.tensor.matmul(psv[:], wv_sb[:, d, :], rhs[d // 4][:, d % 4, rs],
                                     start=(d == 0), stop=(d == DT - 1))
                nc.vector.tensor_scalar_add(vT[:, cs], psv[:], bv_sb[:])
                for tt in range(4 * ci, 4 * ci + 4):
                    tp = tr_ps.tile([P, HD], F32, name="vtr", tag="vtr")
                    nc.tensor.transpose(tp[:], vT[:, tt * P:(tt + 1) * P],
                                        ident[:HD, :HD])
                    nc.vector.tensor_copy(v_aug[:, tt, 0:HD], tp[:])

            def attn_chunk(ci):
                cs = slice(ci * CH, (ci + 1) * CH)
                nkt = 4 * (ci + 1)
                ops = o_ps.tile([HD + 1, CH], F32, name="oacc", tag="oacc")
                for j in range(nkt // 2):
                    ka, kb = 2 * j, 2 * j + 1
                    s2 = s_ps.tile([P, 2 * CH], F32, name="sT", tag="sT")
                    # rows 0-63 of the array: K^T from qk2, Q^T from qk1
                    nc.tensor.matmul(s2[:, 0:CH], qk2[0:HD, ka * P:(ka + 1) * P],
                                     qk1[0:HD, cs], start=True, stop=True)
                    # rows 64-127: K^T from qk1, Q^T from qk2 (concurrent)
                    nc.tensor.matmul(s2[:, CH:2 * CH], qk1[HD:P, kb * P:(kb + 1) * P],
                                     qk2[HD:P, cs], start=True, stop=True)
                    pt = pwork.tile([P, 2 * CH], F16, name="pT", tag="pT")
                    nc.scalar.activation(pt[:], s2[:],
                                         mybir.ActivationFunctionType.Exp)
                    # causal mask: zero P^T where key > query (diagonal band)
                    for half, kt in ((0, ka), (1, kb)):
                        delta = kt * P - ci * CH
                        if delta >= 0:
                            nc.gpsimd.affine_select(
                                out=pt[:, half * CH:(half + 1) * CH],
                                in_=pt[:, half * CH:(half + 1) * CH],
                                compare_op=mybir.AluOpType.is_ge, fill=0.0,
                                base=-delta, pattern=[[1, CH]],
                                channel_multiplier=-1,
                            )
                    nc.tensor.matmul(ops[:], v_aug[:, ka, :], pt[:, 0:CH],
                                     start=(j == 0), stop=False)
                    nc.tensor.matmul(ops[:], v_aug[:, kb, :], pt[:, CH:2 * CH],
                                     start=False, stop=(j == nkt // 2 - 1))
                osb = owork.tile([HD + 1, CH], F32, name="osb", tag="osb")
                nc.vector.tensor_copy(osb[:], ops[:])
                nc.sync.dma_start(out=outT[:, cs], in_=osb[:])

            for ci in range(NCH):
                proj_chunk(ci)
                attn_chunk(ci)

    if legalize:
        _legalize_waits(nc, mybir)
    return nc


def _legalize_waits(nc, mybir):
    """Split multi-wait instructions for the XLA-route walrus codegen.

    The TPB EVENTS struct holds one semaphore wait per instruction and this
    pipeline's codegen refuses >1. Hoist extra waits onto standalone
    EventSemaphore instructions on the same engine queue right before the
    instruction - semantically identical, the queue stalls there.
    """
    n = 0
    for f in nc.m.functions:
        for b in f.blocks:
            out = []
            changed = False
            for inst in b.instructions:
                si = inst.sync_info
                waits = list(si.on_wait) if si is not None and si.on_wait else []
                if len(waits) > 1:
                    changed = True
                    for w in waits[:-1]:
                        n += 1
                        out.append(mybir.InstEventSemaphore(
                            name=f"waitfix{n}_{inst.name}",
                            engine=inst.engine,
                            sync_info=mybir.SyncInfo(on_wait=[w], on_update=[]),
                        ))
                    inst.sync_info = mybir.SyncInfo(
                        on_wait=waits[-1:],
                        on_update=list(si.on_update or []),
                    )
                out.append(inst)
            if changed:
                b.instructions = out
    return n


def kernel(x, wq, bq, wk, bk, wv, bv):
    global LAST_RESULTS
    import os
    os.environ.setdefault("JAX_PLATFORMS", "")
    from concourse.bass_utils import run_bass_kernel_spmd

    x = np.asarray(x, dtype=np.float32)
    s = np.float32(1.0 / np.sqrt(HD))
    wq_s = np.asarray(wq, np.float32) * s
    wk_f = np.asarray(wk, np.float32)
    w1 = np.ascontiguousarray(np.concatenate([wq_s, wk_f], 0).T.astype(np.float16))
    wv_c = np.ascontiguousarray(np.asarray(wv, np.float32).T.astype(np.float16))
    b1 = np.ascontiguousarray(
        np.concatenate([np.asarray(bq, np.float32) * s,
                        np.asarray(bk, np.float32)]).reshape(P, 1))
    bv_c = np.ascontiguousarray(np.asarray(bv, np.float32).reshape(HD, 1))
    xT = np.ascontiguousarray(np.swapaxes(x, 1, 2).astype(np.float16))  # (B, D, T)

    nc = _build_module()
    in_maps = [
        {"xT": xT[b], "w1": w1, "wv": wv_c, "b1": b1, "bv": bv_c}
        for b in range(B)
    ]
    res = run_bass_kernel_spmd(nc, in_maps, core_ids=list(range(B)))
    LAST_RESULTS = res

    out = np.empty((B, T, HD), dtype=np.float32)
    for b in range(B):
        oT = res.results[b]["outT"]  # (65, T): rows 0..63 = O^T, row 64 = denom
        out[b] = (oT[:HD] / oT[HD:HD + 1]).T
    return out
